# revision 1
# baseline (speedup 1.0000x reference)
"""GCN layer (CrossViewGCN layer 1) on 8 Trainium2 NeuronCores.

Reference computation (shapes hardcoded):
    X = input[:, :512]                      # [8192, 512]
    A = input[:, 512:8704] + I              # [8192, 8192]
    d = colsum(A); Dh = diag(d^-1/2)
    support = X @ W                         # [8192, 256]
    out_mm  = Dh @ A @ Dh @ support         # [8192, 256]
    return concat(out_mm, A)                # [8192, 8448]

Sharding: 1D row partition of A / output across the 8 cores (1024 rows
each). The diagonal scalings are folded into the small operands and the
bulk matmul is decomposed around its means so the device does a single
fp8 DoubleRow GEMM per core while all mean terms stay exact f32:

    S    = d^-1/2[:, None] * (X @ W)                  (host, [8192, 256])
    A+I  = a0*J + dA,  S = mu + dS   (a0 = 0.5, mu = colmean(S))
    out_mm rows_i = d^-1/2[rows_i] * ( a0*colsum(S)
                                     + rowsum(dA_i)*mu
                                     + dA_i @ dS )    (device: dA_i @ dS)

dA in [-0.5, 0.5] and dS (zero-mean) are an order of magnitude smaller
than A and S, so quantizing them to fp8e4m3 yields ~7e-6 global
relative error (better than a direct bf16 GEMM) while DoubleRow packs a
256-deep contraction per matmul.

Device-side layouts are partition-major ([128, slab, free]) so every
DMA is 128 long contiguous runs — fast HWDGE descriptor generation.
DMA issue alternates between the two HWDGE rings (SP via nc.sync, ACT
via nc.scalar) so transfers on the two rings overlap.
"""

import numpy as np
import ml_dtypes

NSMP = 8192
NA = 512
DOUT = 256
REALNA = 520
NCORES = 8
ROWS = NSMP // NCORES  # 1024 output rows per core
P = 128
KSLABS = NSMP // P  # 64 contraction slabs of 128
KPAIRS = KSLABS // 2  # 32 DoubleRow slab-pairs (256-deep each)
S_GRP = 8  # k-slabs per S chunk DMA
A_GRP = 8  # k-slabs per streamed dA^T DMA group (1 MiB fp8)
MM_N = 512  # output free dim per matmul (= one PSUM bank of f32)

A0 = np.float32(0.5)  # mean removed from A+I before fp8 quantization

_compiled = None
last_results = None  # BassKernelResults of the most recent run (for harnesses)


def _get_compiled():
    global _compiled
    if _compiled is not None:
        return _compiled

    import concourse.bacc as bacc
    import concourse.mybir as mybir
    import concourse.tile as tile

    fp8 = mybir.dt.float8e4
    f32 = mybir.dt.float32
    DR = mybir.MatmulPerfMode.DoubleRow

    nc = bacc.Bacc(
        "TRN2", target_bir_lowering=False, debug=False, num_devices=NCORES
    )
    # partition-major: at[p, t, m] = dA_i^T[t*128 + p, m]
    at = nc.dram_tensor("at", [P, KSLABS, ROWS], fp8, kind="ExternalInput")
    # partition-major: s[p, t, n] = dS[t*128 + p, n]
    s = nc.dram_tensor("s", [P, KSLABS, DOUT], fp8, kind="ExternalInput")
    ot = nc.dram_tensor("ot", [DOUT, ROWS], f32, kind="ExternalOutput")

    n_schunk = KSLABS // S_GRP  # 8
    # dA groups: small first groups so the PE starts ~2 us after the DMA
    # rings open, then 4-slab (0.5 MiB) groups for fine-grained just-in-time
    # delivery. The A stream alone demands ~260 GB/s against a ~340 GB/s
    # aggregate wire, so delivery order must match consumption order.
    a_groups = [2, 2] + [4] * 15  # slab counts, sum = 64, all even

    with tile.TileContext(nc) as tc:
        with (
            tc.tile_pool(name="s_pool", bufs=1) as s_pool,
            tc.tile_pool(name="a_pool", bufs=10) as a_pool,
            tc.tile_pool(name="o_pool", bufs=1) as o_pool,
            tc.tile_pool(name="ps_pool", bufs=1, space="PSUM") as ps_pool,
        ):
            # dS stays resident: 8 chunks x [128, 8, 256] fp8 (2 MiB total),
            # loaded in consumption order alongside the dA stream.
            s_tiles = [None] * n_schunk

            def load_s_chunk(c, eng):
                s_t = s_pool.tile(
                    [P, S_GRP, DOUT], fp8, name=f"s_t{c}", tag=f"s_t{c}"
                )
                eng.dma_start(out=s_t[:], in_=s[:, c * S_GRP : (c + 1) * S_GRP, :])
                s_tiles[c] = s_t

            # OT = dS^T @ dA^T as two [128, 1024] f32 PSUM tiles
            # (n-tile j covers output rows j*128..(j+1)*128 of ot).
            ps = []
            for j in range(DOUT // P):
                ps_t = ps_pool.tile([P, ROWS], f32, name=f"ps{j}", tag=f"ps{j}")
                ps.append(ps_t)

            # Pre-warm the PE HAM clock gate during the preamble/DMA dead
            # time: ~4.5 us of dependency-free matmuls into a scratch PSUM
            # bank keep the PE busy through one 4096-cycle activity window,
            # so the real stream starts at 2.4 GHz instead of 1.2 GHz.
            warm_in = o_pool.tile([P, 2, MM_N], fp8, name="warm_in", tag="warm_in")
            warm_ps = ps_pool.tile([P, MM_N], f32, name="warm_ps", tag="warm_ps")
            nc.gpsimd.memset(warm_in[:1, :1, :1], 0.0)
            for _ in range(15):
                nc.tensor.matmul(
                    warm_ps[:],
                    warm_in[:, :, :P],
                    warm_in[:],
                    start=True,
                    stop=True,
                    perf_mode=DR,
                )

            # Merge S-chunk and dA-group DMAs into one deadline-ordered list
            # (s_c is consumed starting at slab S_GRP*c), then alternate rings
            # item-by-item: each ring's FIFO is then also deadline-ordered
            # and the two rings share the wire ~evenly.
            items = []
            si = 0
            off = 0
            for u, grp in enumerate(a_groups):
                while si < n_schunk and S_GRP * si < off + grp:
                    items.append(("s", si, 0))
                    si += 1
                items.append(("a", u, off))
                off += grp

            for idx, (kind, u, off) in enumerate(items):
                eng = nc.sync if idx % 2 == 0 else nc.scalar
                if kind == "s":
                    load_s_chunk(u, eng)
                    continue
                grp = a_groups[u]
                a_t = a_pool.tile([P, grp, ROWS], fp8, name="a_t", tag="a_t")
                eng.dma_start(out=a_t[:], in_=at[:, off : off + grp, :])
                for g in range(0, grp, 2):
                    t = off + g  # first slab of the pair
                    q = t // 2  # DoubleRow pair index
                    sc = s_tiles[t // S_GRP]
                    sl = t % S_GRP
                    for j in range(DOUT // P):
                        # [128, 2, 128] stationary: two k-slabs per PE cell
                        lhsT = sc[:, sl : sl + 2, j * P : (j + 1) * P]
                        for mc in range(ROWS // MM_N):
                            nc.tensor.matmul(
                                ps[j][:, mc * MM_N : (mc + 1) * MM_N],
                                lhsT,
                                a_t[:, g : g + 2, mc * MM_N : (mc + 1) * MM_N],
                                start=(q == 0),
                                stop=(q == KPAIRS - 1),
                                perf_mode=DR,
                            )

            # evict per PSUM bank so each copy/DMA overlaps the final matmuls
            # of the other banks
            for j in range(DOUT // P):
                for mc in range(ROWS // MM_N):
                    o_t = o_pool.tile(
                        [P, MM_N], f32, name=f"o_t{j}_{mc}", tag=f"o_t{j}_{mc}"
                    )
                    nc.vector.tensor_copy(
                        o_t[:], ps[j][:, mc * MM_N : (mc + 1) * MM_N]
                    )
                    # spread the final write-backs over both HWDGE rings so
                    # their completion receipts overlap
                    oeng = nc.sync if j == 0 else nc.scalar
                    oeng.dma_start(
                        out=ot[j * P : (j + 1) * P, mc * MM_N : (mc + 1) * MM_N],
                        in_=o_t[:],
                    )

    nc.compile()
    _compiled = nc
    return _compiled


def kernel(input, weight):
    global last_results
    input = np.asarray(input, dtype=np.float32)
    weight = np.asarray(weight, dtype=np.float32)

    X = input[:, :NA]
    A = input[:, REALNA - 8 : REALNA - 8 + NSMP]  # [8192, 8192] view (no +I yet)

    # d = colsum(A + I); the identity adds exactly 1 to every column sum.
    d = A.sum(axis=0, dtype=np.float64) + 1.0
    dinv = (1.0 / np.sqrt(d)).astype(np.float32)  # [8192]
    # rowsum(dA) = rowsum(A + I) - a0*8192, needed for the mean correction
    rowsum_dA = (A.sum(axis=1, dtype=np.float64) + 1.0 - float(A0) * NSMP).astype(
        np.float32
    )

    support = X @ weight  # [8192, 256] f32
    S = support * dinv[:, None]
    mu = S.mean(axis=0, dtype=np.float64).astype(np.float32)  # [256]
    colsum_S = S.sum(axis=0, dtype=np.float64).astype(np.float32)  # [256]
    dS = (S - mu[None, :]).astype(ml_dtypes.float8_e4m3)
    # partition-major [128, 64, 256]
    s_dev = np.ascontiguousarray(dS.reshape(KSLABS, P, DOUT).swapaxes(0, 1))

    diag = np.arange(ROWS)
    in_maps = []
    for i in range(NCORES):
        blk = A[i * ROWS : (i + 1) * ROWS, :]  # [1024, 8192] view
        at_i = (blk.T - A0).astype(ml_dtypes.float8_e4m3)  # [8192, 1024]
        grows = i * ROWS + diag
        # fold the +I into this block's transposed, centered copy
        at_i[grows, diag] = (blk[diag, grows] + (1.0 - A0)).astype(
            ml_dtypes.float8_e4m3
        )
        # partition-major [128, 64, 1024]
        at_dev = np.ascontiguousarray(at_i.reshape(KSLABS, P, ROWS).swapaxes(0, 1))
        in_maps.append({"at": at_dev, "s": s_dev})

    # If BASS_TRACE is set but the axon NTFF hook module is absent, the
    # bass_utils trace path would die on import; provide a no-op hook so it
    # degrades to an untraced run instead.
    try:
        import antenv.axon_hooks  # noqa: F401
    except Exception:
        import sys
        import types

        _m = types.ModuleType("antenv.axon_hooks")
        _m.get_axon_ntff_profile_hook = lambda: None
        _m.set_axon_ntff_profile_hook = lambda h: None
        sys.modules["antenv.axon_hooks"] = _m

    from concourse.bass_utils import run_bass_kernel_spmd

    nc = _get_compiled()
    res = run_bass_kernel_spmd(nc, in_maps, list(range(NCORES)))
    last_results = res

    out = np.empty((NSMP, DOUT + NSMP), dtype=np.float32)
    out[:, DOUT:] = A
    gr = np.arange(NSMP)
    out[gr, DOUT + gr] += 1.0
    # exact mean terms: a0*colsum(S) + rowsum(dA)[:, None] * mu
    mean_terms = float(A0) * colsum_S[None, :] + rowsum_dA[:, None] * mu[None, :]
    for i in range(NCORES):
        ot_i = res.results[i]["ot"]  # [256, 1024] f32 = (dA_i @ dS)^T
        rows = slice(i * ROWS, (i + 1) * ROWS)
        out[rows, :DOUT] = (ot_i.T + mean_terms[rows]) * dinv[rows, None]
    return out



# revision 3
# speedup vs baseline: 1.0040x; 1.0040x over previous
"""GCN layer (CrossViewGCN layer 1) on 8 Trainium2 NeuronCores.

Reference computation (shapes hardcoded):
    X = input[:, :512]                      # [8192, 512]
    A = input[:, 512:8704] + I              # [8192, 8192]
    d = colsum(A); Dh = diag(d^-1/2)
    support = X @ W                         # [8192, 256]
    out_mm  = Dh @ A @ Dh @ support         # [8192, 256]
    return concat(out_mm, A)                # [8192, 8448]

Sharding: 1D row partition of A / output across the 8 cores (1024 rows
each). The diagonal scalings are folded into the small operands and the
bulk matmul is decomposed around its means so the device does a single
fp8 DoubleRow GEMM per core while all mean terms stay exact f32:

    S    = d^-1/2[:, None] * (X @ W)                  (host, [8192, 256])
    A+I  = a0*J + dA,  S = mu + dS   (a0 = 0.5, mu = colmean(S))
    out_mm rows_i = d^-1/2[rows_i] * ( a0*colsum(S)
                                     + rowsum(dA_i)*mu
                                     + dA_i @ dS )    (device: dA_i @ dS)

The device GEMM streams 10.4 MiB of HBM per core (8 MiB dA + 2 MiB dS)
against ~28 us of warm-PE matmul time, so the kernel is delivery-bound:
every scheduling choice below exists to keep the two HWDGE rings wire-
saturated from the first instruction to the last matmul.

 - dA is laid out pass-major on the host ([2, 128, 64, 512] fp8): the
   GEMM runs as two 512-wide column passes, so each pass's chunks are
   fully contiguous HBM reads and the pass-0 output banks evict while
   pass 1 is still streaming (halves the tail).
 - All tiles are SBUF-resident (no pool reuse): every DMA is issued
   up-front in consumption-deadline order, alternating between the SP
   and ACT HWDGE rings, with small leading chunks so the first matmul
   starts ~2 us earlier and ramping to 1 MiB chunks for wire
   efficiency. No backpressure semaphores ever stall a ring.
 - Warmup matmuls bridge the fixed ~7 us Tile/NEFF preamble so the PE
   HAM clock-gate flips to 2.4 GHz (K=8/8) right as real data lands.
 - Within each pass the last 8 pairs run j0-then-j1 so the two PSUM
   banks finish staggered and the copy+writeback of bank j0 overlaps
   the final matmuls of bank j1.
 - The mm-block result goes back as float16 (0.5 MiB instead of 1 MiB
   of f32): the exact mean terms are added on the host in f32, and the
   fluctuation term dA@dS is O(10), so f16 rounding adds ~5e-5
   relative error against a 2e-2 budget.

dA in [-0.5, 0.5] and dS (zero-mean) are an order of magnitude smaller
than A and S, so quantizing them to fp8e4m3 yields ~7e-6 global
relative error while DoubleRow packs a 256-deep contraction per matmul.
"""

import numpy as np
import ml_dtypes

NSMP = 8192
NA = 512
DOUT = 256
REALNA = 520
NCORES = 8
ROWS = NSMP // NCORES  # 1024 output rows per core
P = 128
KSLABS = NSMP // P  # 64 contraction slabs of 128
KPAIRS = KSLABS // 2  # 32 DoubleRow slab-pairs (256-deep each)
S_GRP = 8  # k-slabs per S chunk DMA
HALF = 512  # output free dim per pass (= one PSUM bank of f32)
NPASS = ROWS // HALF  # 2 column passes
N_WARM = 12  # dep-free warmup matmuls (HAM un-throttle + preamble bridge)
TAIL = 8  # pairs per pass run j-separated to stagger PSUM completion

A0 = np.float32(0.5)  # mean removed from A+I before fp8 quantization

# dA chunk sizes in k-slabs per pass: small leading chunks so the first
# pair is ready ~2 us earlier, then 1 MiB chunks for wire efficiency.
A_CHUNKS = [
    [2, 2, 4, 8, 16, 16, 16],  # pass 0 (+ all 8 dS chunks interleave here)
    [16, 16, 16, 16],  # pass 1
]
assert all(sum(c) == KSLABS for c in A_CHUNKS)

_compiled = None
last_results = None  # BassKernelResults of the most recent run (for harnesses)


def _get_compiled():
    global _compiled
    if _compiled is not None:
        return _compiled

    import concourse.bacc as bacc
    import concourse.mybir as mybir
    import concourse.tile as tile

    fp8 = mybir.dt.float8e4
    f16 = mybir.dt.float16
    f32 = mybir.dt.float32
    DR = mybir.MatmulPerfMode.DoubleRow

    nc = bacc.Bacc(
        "TRN2", target_bir_lowering=False, debug=False, num_devices=NCORES
    )
    # pass-major: at[h, p, t, m] = dA_i^T[t*128 + p, h*512 + m]
    at = nc.dram_tensor("at", [NPASS, P, KSLABS, HALF], fp8, kind="ExternalInput")
    # partition-major: s[p, t, n] = dS[t*128 + p, n]
    s = nc.dram_tensor("s", [P, KSLABS, DOUT], fp8, kind="ExternalInput")
    ot = nc.dram_tensor("ot", [DOUT, ROWS], f16, kind="ExternalOutput")

    n_schunk = KSLABS // S_GRP  # 8
    n_achunk = sum(len(c) for c in A_CHUNKS)

    with tile.TileContext(nc) as tc:
        # every tile below has its own tag (all SBUF-resident), so each pool
        # ring is a single buffer per tag
        with (
            tc.tile_pool(name="s_pool", bufs=1) as s_pool,
            tc.tile_pool(name="a_pool", bufs=1) as a_pool,
            tc.tile_pool(name="o_pool", bufs=1) as o_pool,
            tc.tile_pool(name="ps_pool", bufs=1, space="PSUM") as ps_pool,
        ):
            # PE warmup: dependency-free matmuls bridge the fixed preamble so
            # the HAM clock-gate sees a full busy window (-> 2.4 GHz) before
            # the real stream starts, and the PE never idles waiting for the
            # first dA chunk.
            warm_in = o_pool.tile([P, 2, HALF], fp8, name="warm_in", tag="warm_in")
            warm_ps = ps_pool.tile([P, HALF], f32, name="warm_ps", tag="warm_ps")
            nc.gpsimd.memset(warm_in[:1, :1, :1], 0.0)
            for _ in range(N_WARM):
                nc.tensor.matmul(
                    warm_ps[:],
                    warm_in[:, :, :P],
                    warm_in[:],
                    start=True,
                    stop=True,
                    perf_mode=DR,
                )

            # ---- DMA schedule: everything resident, issued up-front ----
            # item = (need_slab, kind, ...); need_slab is the global slab
            # index (pass*64 + t) at which the chunk is first consumed.
            items = []
            for c in range(n_schunk):
                items.append((S_GRP * c, 0, "s", c))
            for h, sizes in enumerate(A_CHUNKS):
                off = 0
                for u, grp in enumerate(sizes):
                    items.append((h * KSLABS + off, 1, "a", (h, u, off, grp)))
                    off += grp
            items.sort()

            s_tiles = [None] * n_schunk
            # slab_map[h][t] -> (tile, local_slab_offset)
            slab_map = [[None] * KSLABS for _ in range(NPASS)]
            for idx, (_, _, kind, info) in enumerate(items):
                eng = nc.sync if idx % 2 == 0 else nc.scalar
                if kind == "s":
                    c = info
                    s_t = s_pool.tile(
                        [P, S_GRP, DOUT], fp8, name=f"s_t{c}", tag=f"s_t{c}"
                    )
                    eng.dma_start(
                        out=s_t[:], in_=s[:, c * S_GRP : (c + 1) * S_GRP, :]
                    )
                    s_tiles[c] = s_t
                else:
                    h, u, off, grp = info
                    a_t = a_pool.tile(
                        [P, grp, HALF], fp8, name=f"a{h}_{u}", tag=f"a{h}_{u}"
                    )
                    eng.dma_start(out=a_t[:], in_=at[h, :, off : off + grp, :])
                    for g in range(grp):
                        slab_map[h][off + g] = (a_t, g)

            # ---- matmul stream: two 512-wide column passes ----
            # Pass h accumulates ps[h][j] (j = output-column tile of ot) over
            # all 32 pairs. The last TAIL pairs run j0-then-j1 so bank j0
            # stops ~1.7 us before bank j1 and its eviction overlaps j1's
            # final matmuls.
            n_out = 0
            for h in range(NPASS):
                ps = [
                    ps_pool.tile([P, HALF], f32, name=f"ps{h}_{j}", tag=f"ps{h}_{j}")
                    for j in range(DOUT // P)
                ]
                order = [(q, j) for q in range(KPAIRS - TAIL) for j in (0, 1)]
                order += [(q, 0) for q in range(KPAIRS - TAIL, KPAIRS)]
                order += [(q, 1) for q in range(KPAIRS - TAIL, KPAIRS)]
                for q, j in order:
                    t = 2 * q  # first slab of the pair
                    a_t, g = slab_map[h][t]
                    sc = s_tiles[t // S_GRP]
                    sl = t % S_GRP
                    nc.tensor.matmul(
                        ps[j][:],
                        sc[:, sl : sl + 2, j * P : (j + 1) * P],
                        a_t[:, g : g + 2, :],
                        start=(q == 0),
                        stop=(q == KPAIRS - 1),
                        perf_mode=DR,
                    )
                    if q == KPAIRS - 1:
                        # bank (h, j) is complete: evict to f16 and write back
                        # while the remaining matmuls keep the PE busy
                        o_t = o_pool.tile(
                            [P, HALF], f16, name=f"o_t{h}_{j}", tag=f"o_t{h}_{j}"
                        )
                        nc.vector.tensor_copy(o_t[:], ps[j][:])
                        oeng = nc.sync if n_out % 2 == 0 else nc.scalar
                        n_out += 1
                        oeng.dma_start(
                            out=ot[
                                j * P : (j + 1) * P, h * HALF : (h + 1) * HALF
                            ],
                            in_=o_t[:],
                        )

    nc.compile()
    _compiled = nc
    return _compiled


def kernel(input, weight):
    global last_results
    input = np.asarray(input, dtype=np.float32)
    weight = np.asarray(weight, dtype=np.float32)

    X = input[:, :NA]
    A = input[:, REALNA - 8 : REALNA - 8 + NSMP]  # [8192, 8192] view (no +I yet)

    # d = colsum(A + I); the identity adds exactly 1 to every column sum.
    d = A.sum(axis=0, dtype=np.float64) + 1.0
    dinv = (1.0 / np.sqrt(d)).astype(np.float32)  # [8192]
    # rowsum(dA) = rowsum(A + I) - a0*8192, needed for the mean correction
    rowsum_dA = (A.sum(axis=1, dtype=np.float64) + 1.0 - float(A0) * NSMP).astype(
        np.float32
    )

    support = X @ weight  # [8192, 256] f32
    S = support * dinv[:, None]
    mu = S.mean(axis=0, dtype=np.float64).astype(np.float32)  # [256]
    colsum_S = S.sum(axis=0, dtype=np.float64).astype(np.float32)  # [256]
    dS = (S - mu[None, :]).astype(ml_dtypes.float8_e4m3)
    # partition-major [128, 64, 256]
    s_dev = np.ascontiguousarray(dS.reshape(KSLABS, P, DOUT).swapaxes(0, 1))

    diag = np.arange(ROWS)
    in_maps = []
    for i in range(NCORES):
        blk = A[i * ROWS : (i + 1) * ROWS, :]  # [1024, 8192] view
        at_i = (blk.T - A0).astype(ml_dtypes.float8_e4m3)  # [8192, 1024]
        grows = i * ROWS + diag
        # fold the +I into this block's transposed, centered copy
        at_i[grows, diag] = (blk[diag, grows] + (1.0 - A0)).astype(
            ml_dtypes.float8_e4m3
        )
        # pass-major [2, 128, 64, 512]: at_dev[h, p, t, m] = at_i[t*128+p, h*512+m]
        at_dev = np.ascontiguousarray(
            at_i.reshape(KSLABS, P, NPASS, HALF).transpose(2, 1, 0, 3)
        )
        in_maps.append({"at": at_dev, "s": s_dev})

    # If BASS_TRACE is set but the axon NTFF hook module is absent, the
    # bass_utils trace path would die on import; provide a no-op hook so it
    # degrades to an untraced run instead.
    try:
        import antenv.axon_hooks  # noqa: F401
    except Exception:
        import sys
        import types

        _m = types.ModuleType("antenv.axon_hooks")
        _m.get_axon_ntff_profile_hook = lambda: None
        _m.set_axon_ntff_profile_hook = lambda h: None
        sys.modules["antenv.axon_hooks"] = _m

    from concourse.bass_utils import run_bass_kernel_spmd

    nc = _get_compiled()
    res = run_bass_kernel_spmd(nc, in_maps, list(range(NCORES)))
    last_results = res

    out = np.empty((NSMP, DOUT + NSMP), dtype=np.float32)
    out[:, DOUT:] = A
    gr = np.arange(NSMP)
    out[gr, DOUT + gr] += 1.0
    # exact mean terms: a0*colsum(S) + rowsum(dA)[:, None] * mu
    mean_terms = float(A0) * colsum_S[None, :] + rowsum_dA[:, None] * mu[None, :]
    for i in range(NCORES):
        ot_i = res.results[i]["ot"]  # [256, 1024] f16 = (dA_i @ dS)^T
        rows = slice(i * ROWS, (i + 1) * ROWS)
        out[rows, :DOUT] = (ot_i.T.astype(np.float32) + mean_terms[rows]) * dinv[
            rows, None
        ]
    return out


# revision 4
# speedup vs baseline: 1.0245x; 1.0204x over previous
"""GCN layer (CrossViewGCN layer 1) on 8 Trainium2 NeuronCores.

Reference computation (shapes hardcoded):
    X = input[:, :512]                      # [8192, 512]
    A = input[:, 512:8704] + I              # [8192, 8192]
    d = colsum(A); Dh = diag(d^-1/2)
    support = X @ W                         # [8192, 256]
    out_mm  = Dh @ A @ Dh @ support         # [8192, 256]
    return concat(out_mm, A)                # [8192, 8448]

Sharding: 1D row partition of A / output across the 8 cores (1024 rows
each). The diagonal scalings are folded into the small operands and the
bulk matmul is decomposed around its means so the device does a single
fp8 DoubleRow GEMM per core while all mean terms stay exact f32:

    S    = d^-1/2[:, None] * (X @ W)                  (host, [8192, 256])
    A+I  = a0*J + dA,  S = mu + dS   (a0 = 0.5, mu = colmean(S))
    out_mm rows_i = d^-1/2[rows_i] * ( a0*colsum(S)
                                     + rowsum(dA_i)*mu
                                     + dA_i @ dS )    (device: dA_i @ dS)

Measured on hardware (NTFF packet records): the two HWDGE rings sustain
~420 GB/s per core once ramped, so the 10.4 MiB HBM stream (8 MiB dA +
2 MiB dS) lands well before the PE can consume it — the kernel is
PE-bound at 128 warm DoubleRow matmuls x 216 ns = 27.6 us. The
scheduling below exists to keep the PE at exactly that pace:

 - All tiles are SBUF-resident (no pool reuse, no backpressure): every
   DMA is issued up-front in consumption-deadline order, alternating
   rings, with small leading chunks (first pair ready ~10.5 us) and
   0.5 MiB steady-state chunks so tile-ready semaphores release work in
   fine quanta (a chunk's sem fires ~2 us after its last byte lands -
   coarse chunks were measured to stall the PE 1-3 us and re-throttle
   the HAM clock gate).
 - Warmup matmuls bridge the fixed ~7 us Tile/NEFF preamble so the HAM
   clock-gate flips to 2.4 GHz (K=8/8) before the real stream starts
   and the PE never sees a >3.4 us idle window after that.
 - The last 8 pairs run grouped per PSUM bank (all of bank j0/mc0's
   matmuls, then j0/mc1, ...) so the four banks stop staggered ~1.7 us
   apart and three of the four copy+writeback chains overlap the final
   matmuls; only the last bank's ~3 us eviction is exposed.
 - The mm-block result goes back as float16 (0.5 MiB instead of 1 MiB):
   the exact mean terms are added on the host in f32, and the
   fluctuation term dA@dS is O(10), so f16 rounding adds ~5e-5 relative
   error against a 2e-2 budget.

dA in [-0.5, 0.5] and dS (zero-mean) are an order of magnitude smaller
than A and S, so quantizing them to fp8e4m3 yields ~7e-6 global
relative error while DoubleRow packs a 256-deep contraction per matmul.
"""

import numpy as np
import ml_dtypes

NSMP = 8192
NA = 512
DOUT = 256
REALNA = 520
NCORES = 8
ROWS = NSMP // NCORES  # 1024 output rows per core
P = 128
KSLABS = NSMP // P  # 64 contraction slabs of 128
KPAIRS = KSLABS // 2  # 32 DoubleRow slab-pairs (256-deep each)
S_GRP = 8  # k-slabs per S chunk DMA
MM_N = 512  # output free dim per matmul (= one PSUM bank of f32)
N_WARM = 10  # dep-free warmup matmuls (HAM un-throttle + preamble bridge)
TAIL = 8  # pairs run bank-grouped at the end to stagger PSUM completion

A0 = np.float32(0.5)  # mean removed from A+I before fp8 quantization

# dA chunk sizes in k-slabs: small leading chunks so the first pair is
# ready early, then 0.5 MiB chunks (fine-grained tile-ready sems).
A_CHUNKS = [2, 2, 2, 2] + [4] * 14
assert sum(A_CHUNKS) == KSLABS and all(c % 2 == 0 for c in A_CHUNKS)

_compiled = None
last_results = None  # BassKernelResults of the most recent run (for harnesses)


def _get_compiled():
    global _compiled
    if _compiled is not None:
        return _compiled

    import concourse.bacc as bacc
    import concourse.mybir as mybir
    import concourse.tile as tile

    fp8 = mybir.dt.float8e4
    f16 = mybir.dt.float16
    f32 = mybir.dt.float32
    DR = mybir.MatmulPerfMode.DoubleRow

    nc = bacc.Bacc(
        "TRN2", target_bir_lowering=False, debug=False, num_devices=NCORES
    )
    # partition-major: at[p, t, m] = dA_i^T[t*128 + p, m]
    at = nc.dram_tensor("at", [P, KSLABS, ROWS], fp8, kind="ExternalInput")
    # partition-major: s[p, t, n] = dS[t*128 + p, n]
    s = nc.dram_tensor("s", [P, KSLABS, DOUT], fp8, kind="ExternalInput")
    ot = nc.dram_tensor("ot", [DOUT, ROWS], f16, kind="ExternalOutput")

    n_schunk = KSLABS // S_GRP  # 8

    with tile.TileContext(nc) as tc:
        # every tile below has its own tag (all SBUF-resident), so each pool
        # ring is a single buffer per tag
        with (
            tc.tile_pool(name="s_pool", bufs=1) as s_pool,
            tc.tile_pool(name="a_pool", bufs=1) as a_pool,
            tc.tile_pool(name="o_pool", bufs=1) as o_pool,
            tc.tile_pool(name="ps_pool", bufs=1, space="PSUM") as ps_pool,
        ):
            # PE warmup: dependency-free matmuls bridge the fixed preamble so
            # the HAM clock-gate sees a full busy window (-> 2.4 GHz) before
            # the real stream starts.
            warm_in = o_pool.tile([P, 2, MM_N], fp8, name="warm_in", tag="warm_in")
            warm_ps = ps_pool.tile([P, MM_N], f32, name="warm_ps", tag="warm_ps")
            nc.gpsimd.memset(warm_in[:1, :1, :1], 0.0)
            for _ in range(N_WARM):
                nc.tensor.matmul(
                    warm_ps[:],
                    warm_in[:, :, :P],
                    warm_in[:],
                    start=True,
                    stop=True,
                    perf_mode=DR,
                )

            # ---- DMA schedule: everything resident, issued up-front in
            # consumption-deadline order (need_slab, s-chunks first on ties),
            # alternating the two HWDGE rings item by item ----
            items = []
            for c in range(n_schunk):
                items.append((S_GRP * c, 0, "s", c))
            off = 0
            for u, grp in enumerate(A_CHUNKS):
                items.append((off, 1, "a", (u, off, grp)))
                off += grp
            items.sort()

            s_tiles = [None] * n_schunk
            slab_map = [None] * KSLABS  # slab t -> (tile, local_offset)
            for idx, (_, _, kind, info) in enumerate(items):
                eng = nc.sync if idx % 2 == 0 else nc.scalar
                if kind == "s":
                    c = info
                    s_t = s_pool.tile(
                        [P, S_GRP, DOUT], fp8, name=f"s_t{c}", tag=f"s_t{c}"
                    )
                    eng.dma_start(
                        out=s_t[:], in_=s[:, c * S_GRP : (c + 1) * S_GRP, :]
                    )
                    s_tiles[c] = s_t
                else:
                    u, off, grp = info
                    a_t = a_pool.tile(
                        [P, grp, ROWS], fp8, name=f"a_t{u}", tag=f"a_t{u}"
                    )
                    eng.dma_start(out=a_t[:], in_=at[:, off : off + grp, :])
                    for g in range(grp):
                        slab_map[off + g] = (a_t, g)

            # ---- matmul stream ----
            # OT = dS^T @ dA^T accumulated in four PSUM banks: bank (j, mc)
            # covers ot rows j*128..(j+1)*128, cols mc*512..(mc+1)*512.
            # Body pairs rotate j0mc0, j0mc1, j1mc0, j1mc1 (each lhsT serves
            # two consecutive matmuls); the last TAIL pairs run grouped per
            # bank so the banks stop staggered and evictions overlap the
            # remaining matmuls.
            ps = []
            for j in range(DOUT // P):
                ps_t = ps_pool.tile([P, ROWS], f32, name=f"ps{j}", tag=f"ps{j}")
                ps.append(ps_t)

            def mm(q, j, mc):
                t = 2 * q  # first slab of the pair
                a_t, g = slab_map[t]
                sc = s_tiles[t // S_GRP]
                sl = t % S_GRP
                nc.tensor.matmul(
                    ps[j][:, mc * MM_N : (mc + 1) * MM_N],
                    sc[:, sl : sl + 2, j * P : (j + 1) * P],
                    a_t[:, g : g + 2, mc * MM_N : (mc + 1) * MM_N],
                    start=(q == 0),
                    stop=(q == KPAIRS - 1),
                    perf_mode=DR,
                )

            def evict(j, mc, n_out):
                o_t = o_pool.tile(
                    [P, MM_N], f16, name=f"o_t{j}_{mc}", tag=f"o_t{j}_{mc}"
                )
                nc.vector.tensor_copy(o_t[:], ps[j][:, mc * MM_N : (mc + 1) * MM_N])
                oeng = nc.sync if n_out % 2 == 0 else nc.scalar
                oeng.dma_start(
                    out=ot[j * P : (j + 1) * P, mc * MM_N : (mc + 1) * MM_N],
                    in_=o_t[:],
                )

            for q in range(KPAIRS - TAIL):
                for j in range(DOUT // P):
                    for mc in range(ROWS // MM_N):
                        mm(q, j, mc)
            n_out = 0
            for j in range(DOUT // P):
                for mc in range(ROWS // MM_N):
                    for q in range(KPAIRS - TAIL, KPAIRS):
                        mm(q, j, mc)
                    # bank (j, mc) is complete: evict to f16 and write back
                    # while the remaining banks' matmuls keep the PE busy
                    evict(j, mc, n_out)
                    n_out += 1

    nc.compile()
    _compiled = nc
    return _compiled


def kernel(input, weight):
    global last_results
    input = np.asarray(input, dtype=np.float32)
    weight = np.asarray(weight, dtype=np.float32)

    X = input[:, :NA]
    A = input[:, REALNA - 8 : REALNA - 8 + NSMP]  # [8192, 8192] view (no +I yet)

    # d = colsum(A + I); the identity adds exactly 1 to every column sum.
    d = A.sum(axis=0, dtype=np.float64) + 1.0
    dinv = (1.0 / np.sqrt(d)).astype(np.float32)  # [8192]
    # rowsum(dA) = rowsum(A + I) - a0*8192, needed for the mean correction
    rowsum_dA = (A.sum(axis=1, dtype=np.float64) + 1.0 - float(A0) * NSMP).astype(
        np.float32
    )

    support = X @ weight  # [8192, 256] f32
    S = support * dinv[:, None]
    mu = S.mean(axis=0, dtype=np.float64).astype(np.float32)  # [256]
    colsum_S = S.sum(axis=0, dtype=np.float64).astype(np.float32)  # [256]
    dS = (S - mu[None, :]).astype(ml_dtypes.float8_e4m3)
    # partition-major [128, 64, 256]
    s_dev = np.ascontiguousarray(dS.reshape(KSLABS, P, DOUT).swapaxes(0, 1))

    diag = np.arange(ROWS)
    in_maps = []
    for i in range(NCORES):
        blk = A[i * ROWS : (i + 1) * ROWS, :]  # [1024, 8192] view
        at_i = (blk.T - A0).astype(ml_dtypes.float8_e4m3)  # [8192, 1024]
        grows = i * ROWS + diag
        # fold the +I into this block's transposed, centered copy
        at_i[grows, diag] = (blk[diag, grows] + (1.0 - A0)).astype(
            ml_dtypes.float8_e4m3
        )
        # partition-major [128, 64, 1024]
        at_dev = np.ascontiguousarray(at_i.reshape(KSLABS, P, ROWS).swapaxes(0, 1))
        in_maps.append({"at": at_dev, "s": s_dev})

    # If BASS_TRACE is set but the axon NTFF hook module is absent, the
    # bass_utils trace path would die on import; provide a no-op hook so it
    # degrades to an untraced run instead.
    try:
        import antenv.axon_hooks  # noqa: F401
    except Exception:
        import sys
        import types

        _m = types.ModuleType("antenv.axon_hooks")
        _m.get_axon_ntff_profile_hook = lambda: None
        _m.set_axon_ntff_profile_hook = lambda h: None
        sys.modules["antenv.axon_hooks"] = _m

    from concourse.bass_utils import run_bass_kernel_spmd

    nc = _get_compiled()
    res = run_bass_kernel_spmd(nc, in_maps, list(range(NCORES)))
    last_results = res

    out = np.empty((NSMP, DOUT + NSMP), dtype=np.float32)
    out[:, DOUT:] = A
    gr = np.arange(NSMP)
    out[gr, DOUT + gr] += 1.0
    # exact mean terms: a0*colsum(S) + rowsum(dA)[:, None] * mu
    mean_terms = float(A0) * colsum_S[None, :] + rowsum_dA[:, None] * mu[None, :]
    for i in range(NCORES):
        ot_i = res.results[i]["ot"]  # [256, 1024] f16 = (dA_i @ dS)^T
        rows = slice(i * ROWS, (i + 1) * ROWS)
        out[rows, :DOUT] = (ot_i.T.astype(np.float32) + mean_terms[rows]) * dinv[
            rows, None
        ]
    return out


# revision 8
# speedup vs baseline: 1.0427x; 1.0178x over previous
"""GCN layer (CrossViewGCN layer 1) on 8 Trainium2 NeuronCores.

Reference computation (shapes hardcoded):
    X = input[:, :512]                      # [8192, 512]
    A = input[:, 512:8704] + I              # [8192, 8192]
    d = colsum(A); Dh = diag(d^-1/2)
    support = X @ W                         # [8192, 256]
    out_mm  = Dh @ A @ Dh @ support         # [8192, 256]
    return concat(out_mm, A)                # [8192, 8448]

Sharding: 1D row partition of A / output across the 8 cores (1024 rows
each). The diagonal scalings are folded into the small operands and the
bulk matmul is decomposed around its means so the device does a single
fp8 DoubleRow GEMM per core while all mean terms stay exact f32:

    S    = d^-1/2[:, None] * (X @ W)                  (host, [8192, 256])
    A+I  = a0*J + dA,  S = mu + dS   (a0 = 0.5, mu = colmean(S))
    out_mm rows_i = d^-1/2[rows_i] * ( a0*colsum(S)
                                     + rowsum(dA_i)*mu
                                     + dA_i @ dS )    (device: dA_i @ dS)

Measured on hardware (NTFF packet records): the two HWDGE rings sustain
~420 GB/s per core once ramped, so the 10.4 MiB HBM stream (8 MiB dA +
2 MiB dS) lands well before the PE can consume it — the kernel is
PE-bound at 128 warm DoubleRow matmuls x 216 ns = 27.6 us. The
scheduling below exists to keep the PE at exactly that pace:

 - All tiles are SBUF-resident (no pool reuse, no backpressure): every
   DMA is issued up-front in consumption-deadline order, alternating
   rings, with small leading chunks (first pair ready ~10.5 us) and
   0.5 MiB steady-state chunks so tile-ready semaphores release work in
   fine quanta (a chunk's sem fires ~2 us after its last byte lands -
   coarse chunks were measured to stall the PE 1-3 us and re-throttle
   the HAM clock gate).
 - Warmup matmuls bridge the fixed ~7 us Tile/NEFF preamble so the HAM
   clock-gate flips to 2.4 GHz (K=8/8) before the real stream starts
   and the PE never sees a >3.4 us idle window after that.
 - The last 8 pairs run grouped per PSUM bank (all of bank j0/mc0's
   matmuls, then j0/mc1, ...) so the four banks stop staggered ~1.7 us
   apart and three of the four copy+writeback chains overlap the final
   matmuls; only the last bank's ~3 us eviction is exposed.
 - The mm-block result goes back as float16 (0.5 MiB instead of 1 MiB):
   the exact mean terms are added on the host in f32, and the
   fluctuation term dA@dS is O(10), so f16 rounding adds ~5e-5 relative
   error against a 2e-2 budget.

dA in [-0.5, 0.5] and dS (zero-mean) are an order of magnitude smaller
than A and S, so quantizing them to fp8e4m3 yields ~7e-6 global
relative error while DoubleRow packs a 256-deep contraction per matmul.
"""

import numpy as np
import ml_dtypes

NSMP = 8192
NA = 512
DOUT = 256
REALNA = 520
NCORES = 8
ROWS = NSMP // NCORES  # 1024 output rows per core
P = 128
KSLABS = NSMP // P  # 64 contraction slabs of 128
KPAIRS = KSLABS // 2  # 32 DoubleRow slab-pairs (256-deep each)
MM_N = 512  # output free dim per matmul (= one PSUM bank of f32)
N_WARM = 14  # dep-free warmup matmuls (HAM un-throttle + preamble bridge)
TAIL = 8  # pairs run bank-grouped at the end to stagger PSUM completion

A0 = np.float32(0.5)  # mean removed from A+I before fp8 quantization

# Chunk sizes in k-slabs. A chunk's tile-ready sem fires ~2 us after its
# last byte lands, so the profile tapers: moderate lead-in (ready right
# as the warmups end), 1 MiB mid-stream chunks for wire efficiency, and
# small trailing chunks so the final pairs' sems fire promptly after
# their bytes instead of 2 us after a 1 MiB block.
A_CHUNKS = [4, 4, 8, 8, 8, 8, 8, 4, 4, 4, 2, 2]
S_CHUNKS = [8, 8, 16, 16, 16]
assert sum(A_CHUNKS) == KSLABS and all(c % 2 == 0 for c in A_CHUNKS)
assert sum(S_CHUNKS) == KSLABS and all(c % 8 == 0 for c in S_CHUNKS)

_compiled = None
last_results = None  # BassKernelResults of the most recent run (for harnesses)


def _get_compiled():
    global _compiled
    if _compiled is not None:
        return _compiled

    import concourse.bacc as bacc
    import concourse.mybir as mybir
    import concourse.tile as tile

    fp8 = mybir.dt.float8e4
    f16 = mybir.dt.float16
    f32 = mybir.dt.float32
    DR = mybir.MatmulPerfMode.DoubleRow

    nc = bacc.Bacc(
        "TRN2", target_bir_lowering=False, debug=False, num_devices=NCORES
    )
    # partition-major: at[p, t, m] = dA_i^T[t*128 + p, m]
    at = nc.dram_tensor("at", [P, KSLABS, ROWS], fp8, kind="ExternalInput")
    # partition-major: s[p, t, n] = dS[t*128 + p, n]
    s = nc.dram_tensor("s", [P, KSLABS, DOUT], fp8, kind="ExternalInput")
    ot = nc.dram_tensor("ot", [DOUT, ROWS], f16, kind="ExternalOutput")

    with tile.TileContext(nc) as tc:
        # every tile below has its own tag (all SBUF-resident), so each pool
        # ring is a single buffer per tag
        with (
            tc.tile_pool(name="s_pool", bufs=1) as s_pool,
            tc.tile_pool(name="a_pool", bufs=1) as a_pool,
            tc.tile_pool(name="o_pool", bufs=1) as o_pool,
            tc.tile_pool(name="ps_pool", bufs=1, space="PSUM") as ps_pool,
        ):
            # PE warmup: dependency-free matmuls bridge the fixed preamble so
            # the HAM clock-gate sees a full busy window (-> 2.4 GHz) before
            # the real stream starts.
            warm_in = o_pool.tile([P, 2, MM_N], fp8, name="warm_in", tag="warm_in")
            warm_ps = ps_pool.tile([P, MM_N], f32, name="warm_ps", tag="warm_ps")
            nc.gpsimd.memset(warm_in[:1, :1, :1], 0.0)
            for _ in range(N_WARM):
                nc.tensor.matmul(
                    warm_ps[:],
                    warm_in[:, :, :P],
                    warm_in[:],
                    start=True,
                    stop=True,
                    perf_mode=DR,
                )

            # ---- DMA schedule: everything resident, issued up-front in
            # consumption-deadline order (need_slab, s-chunks first on ties),
            # alternating the two HWDGE rings item by item ----
            items = []
            off = 0
            for u, grp in enumerate(S_CHUNKS):
                items.append((off, 0, "s", (u, off, grp)))
                off += grp
            off = 0
            for u, grp in enumerate(A_CHUNKS):
                items.append((off, 1, "a", (u, off, grp)))
                off += grp
            items.sort()

            s_map = [None] * KSLABS  # slab t -> (tile, local_offset)
            slab_map = [None] * KSLABS  # slab t -> (tile, local_offset)
            for idx, (_, _, kind, info) in enumerate(items):
                eng = nc.sync if idx % 2 == 0 else nc.scalar
                u, off, grp = info
                if kind == "s":
                    s_t = s_pool.tile(
                        [P, grp, DOUT], fp8, name=f"s_t{u}", tag=f"s_t{u}"
                    )
                    eng.dma_start(out=s_t[:], in_=s[:, off : off + grp, :])
                    for g in range(grp):
                        s_map[off + g] = (s_t, g)
                else:
                    a_t = a_pool.tile(
                        [P, grp, ROWS], fp8, name=f"a_t{u}", tag=f"a_t{u}"
                    )
                    eng.dma_start(out=a_t[:], in_=at[:, off : off + grp, :])
                    for g in range(grp):
                        slab_map[off + g] = (a_t, g)

            # ---- matmul stream ----
            # OT = dS^T @ dA^T accumulated in four PSUM banks: bank (j, mc)
            # covers ot rows j*128..(j+1)*128, cols mc*512..(mc+1)*512.
            # Body pairs rotate j0mc0, j0mc1, j1mc0, j1mc1 (each lhsT serves
            # two consecutive matmuls); the last TAIL pairs run grouped per
            # bank so the banks stop staggered and evictions overlap the
            # remaining matmuls.
            ps = []
            for j in range(DOUT // P):
                ps_t = ps_pool.tile([P, ROWS], f32, name=f"ps{j}", tag=f"ps{j}")
                ps.append(ps_t)

            def mm(q, j, mc):
                t = 2 * q  # first slab of the pair
                a_t, g = slab_map[t]
                sc, sl = s_map[t]
                nc.tensor.matmul(
                    ps[j][:, mc * MM_N : (mc + 1) * MM_N],
                    sc[:, sl : sl + 2, j * P : (j + 1) * P],
                    a_t[:, g : g + 2, mc * MM_N : (mc + 1) * MM_N],
                    start=(q == 0),
                    stop=(q == KPAIRS - 1),
                    perf_mode=DR,
                )

            def evict(j, mc, n_out):
                o_t = o_pool.tile(
                    [P, MM_N], f16, name=f"o_t{j}_{mc}", tag=f"o_t{j}_{mc}"
                )
                nc.vector.tensor_copy(o_t[:], ps[j][:, mc * MM_N : (mc + 1) * MM_N])
                oeng = nc.sync if n_out % 2 == 0 else nc.scalar
                oeng.dma_start(
                    out=ot[j * P : (j + 1) * P, mc * MM_N : (mc + 1) * MM_N],
                    in_=o_t[:],
                )

            for q in range(KPAIRS - TAIL):
                for j in range(DOUT // P):
                    for mc in range(ROWS // MM_N):
                        mm(q, j, mc)
            n_out = 0
            for j in range(DOUT // P):
                for mc in range(ROWS // MM_N):
                    for q in range(KPAIRS - TAIL, KPAIRS):
                        mm(q, j, mc)
                    # bank (j, mc) is complete: evict to f16 and write back
                    # while the remaining banks' matmuls keep the PE busy
                    evict(j, mc, n_out)
                    n_out += 1

    nc.compile()
    _compiled = nc
    return _compiled


def kernel(input, weight):
    global last_results
    input = np.asarray(input, dtype=np.float32)
    weight = np.asarray(weight, dtype=np.float32)

    X = input[:, :NA]
    A = input[:, REALNA - 8 : REALNA - 8 + NSMP]  # [8192, 8192] view (no +I yet)

    # d = colsum(A + I); the identity adds exactly 1 to every column sum.
    d = A.sum(axis=0, dtype=np.float64) + 1.0
    dinv = (1.0 / np.sqrt(d)).astype(np.float32)  # [8192]
    # rowsum(dA) = rowsum(A + I) - a0*8192, needed for the mean correction
    rowsum_dA = (A.sum(axis=1, dtype=np.float64) + 1.0 - float(A0) * NSMP).astype(
        np.float32
    )

    support = X @ weight  # [8192, 256] f32
    S = support * dinv[:, None]
    mu = S.mean(axis=0, dtype=np.float64).astype(np.float32)  # [256]
    colsum_S = S.sum(axis=0, dtype=np.float64).astype(np.float32)  # [256]
    dS = (S - mu[None, :]).astype(ml_dtypes.float8_e4m3)
    # partition-major [128, 64, 256]
    s_dev = np.ascontiguousarray(dS.reshape(KSLABS, P, DOUT).swapaxes(0, 1))

    diag = np.arange(ROWS)
    in_maps = []
    for i in range(NCORES):
        blk = A[i * ROWS : (i + 1) * ROWS, :]  # [1024, 8192] view
        at_i = (blk.T - A0).astype(ml_dtypes.float8_e4m3)  # [8192, 1024]
        grows = i * ROWS + diag
        # fold the +I into this block's transposed, centered copy
        at_i[grows, diag] = (blk[diag, grows] + (1.0 - A0)).astype(
            ml_dtypes.float8_e4m3
        )
        # partition-major [128, 64, 1024]
        at_dev = np.ascontiguousarray(at_i.reshape(KSLABS, P, ROWS).swapaxes(0, 1))
        in_maps.append({"at": at_dev, "s": s_dev})

    # If BASS_TRACE is set but the axon NTFF hook module is absent, the
    # bass_utils trace path would die on import; provide a no-op hook so it
    # degrades to an untraced run instead.
    try:
        import antenv.axon_hooks  # noqa: F401
    except Exception:
        import sys
        import types

        _m = types.ModuleType("antenv.axon_hooks")
        _m.get_axon_ntff_profile_hook = lambda: None
        _m.set_axon_ntff_profile_hook = lambda h: None
        sys.modules["antenv.axon_hooks"] = _m

    from concourse.bass_utils import run_bass_kernel_spmd

    nc = _get_compiled()
    res = run_bass_kernel_spmd(nc, in_maps, list(range(NCORES)))
    last_results = res

    out = np.empty((NSMP, DOUT + NSMP), dtype=np.float32)
    out[:, DOUT:] = A
    gr = np.arange(NSMP)
    out[gr, DOUT + gr] += 1.0
    # exact mean terms: a0*colsum(S) + rowsum(dA)[:, None] * mu
    mean_terms = float(A0) * colsum_S[None, :] + rowsum_dA[:, None] * mu[None, :]
    for i in range(NCORES):
        ot_i = res.results[i]["ot"]  # [256, 1024] f16 = (dA_i @ dS)^T
        rows = slice(i * ROWS, (i + 1) * ROWS)
        out[rows, :DOUT] = (ot_i.T.astype(np.float32) + mean_terms[rows]) * dinv[
            rows, None
        ]
    return out


# revision 10
# speedup vs baseline: 1.0499x; 1.0069x over previous
"""GCN layer (CrossViewGCN layer 1) on 8 Trainium2 NeuronCores.

Reference computation (shapes hardcoded):
    X = input[:, :512]                      # [8192, 512]
    A = input[:, 512:8704] + I              # [8192, 8192]
    d = colsum(A); Dh = diag(d^-1/2)
    support = X @ W                         # [8192, 256]
    out_mm  = Dh @ A @ Dh @ support         # [8192, 256]
    return concat(out_mm, A)                # [8192, 8448]

Sharding: 1D row partition of A / output across the 8 cores (1024 rows
each). The diagonal scalings are folded into the small operands and the
bulk matmul is decomposed around its means so the device does a single
fp8 DoubleRow GEMM per core while all mean terms stay exact f32:

    S    = d^-1/2[:, None] * (X @ W)                  (host, [8192, 256])
    A+I  = a0*J + dA,  S = mu + dS   (a0 = 0.5, mu = colmean(S))
    out_mm rows_i = d^-1/2[rows_i] * ( a0*colsum(S)
                                     + rowsum(dA_i)*mu
                                     + dA_i @ dS )    (device: dA_i @ dS)

Measured on hardware (NTFF packet records): the two HWDGE rings sustain
~420 GB/s per core once ramped, so the 10.4 MiB HBM stream (8 MiB dA +
2 MiB dS) lands well before the PE can consume it — the kernel is
PE-bound at 128 warm DoubleRow matmuls x 216 ns = 27.6 us. The
scheduling below exists to keep the PE at exactly that pace:

 - All tiles are SBUF-resident (no pool reuse, no backpressure): every
   DMA is issued up-front in consumption-deadline order, alternating
   rings, with small leading chunks (first pair ready ~10.5 us) and
   0.5 MiB steady-state chunks so tile-ready semaphores release work in
   fine quanta (a chunk's sem fires ~2 us after its last byte lands -
   coarse chunks were measured to stall the PE 1-3 us and re-throttle
   the HAM clock gate).
 - Warmup matmuls bridge the fixed ~7 us Tile/NEFF preamble so the HAM
   clock-gate flips to 2.4 GHz (K=8/8) before the real stream starts
   and the PE never sees a >3.4 us idle window after that.
 - The last 8 pairs run grouped per PSUM bank (all of bank j0/mc0's
   matmuls, then j0/mc1, ...) so the four banks stop staggered ~1.7 us
   apart and three of the four copy+writeback chains overlap the final
   matmuls; only the last bank's ~3 us eviction is exposed.
 - The mm-block result goes back as float16 (0.5 MiB instead of 1 MiB):
   the exact mean terms are added on the host in f32, and the
   fluctuation term dA@dS is O(10), so f16 rounding adds ~5e-5 relative
   error against a 2e-2 budget.

dA in [-0.5, 0.5] and dS (zero-mean) are an order of magnitude smaller
than A and S, so quantizing them to fp8e4m3 yields ~7e-6 global
relative error while DoubleRow packs a 256-deep contraction per matmul.
"""

import numpy as np
import ml_dtypes

NSMP = 8192
NA = 512
DOUT = 256
REALNA = 520
NCORES = 8
ROWS = NSMP // NCORES  # 1024 output rows per core
P = 128
KSLABS = NSMP // P  # 64 contraction slabs of 128
KPAIRS = KSLABS // 2  # 32 DoubleRow slab-pairs (256-deep each)
MM_N = 512  # output free dim per matmul (= one PSUM bank of f32)
N_WARM = 20  # dep-free warmup matmuls (HAM un-throttle + cushion builder)
TAIL = 8  # pairs run bank-grouped at the end to stagger PSUM completion

A0 = np.float32(0.5)  # mean removed from A+I before fp8 quantization

# Chunk sizes in k-slabs. Total wire demand (10.4 MiB / 27.6 us of PE
# time ~= 376 GB/s) sits only ~8% under the measured ~410 GB/s wire, and
# a chunk's tile-ready sem fires ~2 us after its last byte lands. So the
# warmups above deliberately delay the first real matmul to ~13 us,
# building a ~2 MiB SBUF cushion that the stream then never drains —
# without it the PE rides the sem frontier and stalls 1-3 us at a time.
# 1 MiB chunks carry the bulk; trailing chunks shrink so the final
# pairs' sems fire promptly after their bytes.
A_CHUNKS = [8, 8, 8, 8, 8, 8, 4, 4, 4, 2, 2]
S_CHUNKS = [8, 8, 16, 16, 16]
assert sum(A_CHUNKS) == KSLABS and all(c % 2 == 0 for c in A_CHUNKS)
assert sum(S_CHUNKS) == KSLABS and all(c % 8 == 0 for c in S_CHUNKS)

_compiled = None
last_results = None  # BassKernelResults of the most recent run (for harnesses)


def _get_compiled():
    global _compiled
    if _compiled is not None:
        return _compiled

    import concourse.bacc as bacc
    import concourse.mybir as mybir
    import concourse.tile as tile

    fp8 = mybir.dt.float8e4
    f16 = mybir.dt.float16
    f32 = mybir.dt.float32
    DR = mybir.MatmulPerfMode.DoubleRow

    nc = bacc.Bacc(
        "TRN2", target_bir_lowering=False, debug=False, num_devices=NCORES
    )
    # partition-major: at[p, t, m] = dA_i^T[t*128 + p, m]
    at = nc.dram_tensor("at", [P, KSLABS, ROWS], fp8, kind="ExternalInput")
    # partition-major: s[p, t, n] = dS[t*128 + p, n]
    s = nc.dram_tensor("s", [P, KSLABS, DOUT], fp8, kind="ExternalInput")
    ot = nc.dram_tensor("ot", [DOUT, ROWS], f16, kind="ExternalOutput")

    with tile.TileContext(nc) as tc:
        # every tile below has its own tag (all SBUF-resident), so each pool
        # ring is a single buffer per tag
        with (
            tc.tile_pool(name="s_pool", bufs=1) as s_pool,
            tc.tile_pool(name="a_pool", bufs=1) as a_pool,
            tc.tile_pool(name="o_pool", bufs=1) as o_pool,
            tc.tile_pool(name="ps_pool", bufs=1, space="PSUM") as ps_pool,
        ):
            # PE warmup: dependency-free matmuls bridge the fixed preamble so
            # the HAM clock-gate sees a full busy window (-> 2.4 GHz) before
            # the real stream starts.
            warm_in = o_pool.tile([P, 2, MM_N], fp8, name="warm_in", tag="warm_in")
            warm_ps = ps_pool.tile([P, MM_N], f32, name="warm_ps", tag="warm_ps")
            nc.gpsimd.memset(warm_in[:1, :1, :1], 0.0)
            for _ in range(N_WARM):
                nc.tensor.matmul(
                    warm_ps[:],
                    warm_in[:, :, :P],
                    warm_in[:],
                    start=True,
                    stop=True,
                    perf_mode=DR,
                )

            # ---- DMA schedule: everything resident, issued up-front in
            # consumption-deadline order (need_slab, s-chunks first on ties),
            # alternating the two HWDGE rings item by item ----
            items = []
            off = 0
            for u, grp in enumerate(S_CHUNKS):
                items.append((off, 0, "s", (u, off, grp)))
                off += grp
            off = 0
            for u, grp in enumerate(A_CHUNKS):
                items.append((off, 1, "a", (u, off, grp)))
                off += grp
            items.sort()

            s_map = [None] * KSLABS  # slab t -> (tile, local_offset)
            slab_map = [None] * KSLABS  # slab t -> (tile, local_offset)
            for idx, (_, _, kind, info) in enumerate(items):
                eng = nc.sync if idx % 2 == 0 else nc.scalar
                u, off, grp = info
                if kind == "s":
                    s_t = s_pool.tile(
                        [P, grp, DOUT], fp8, name=f"s_t{u}", tag=f"s_t{u}"
                    )
                    eng.dma_start(out=s_t[:], in_=s[:, off : off + grp, :])
                    for g in range(grp):
                        s_map[off + g] = (s_t, g)
                else:
                    a_t = a_pool.tile(
                        [P, grp, ROWS], fp8, name=f"a_t{u}", tag=f"a_t{u}"
                    )
                    eng.dma_start(out=a_t[:], in_=at[:, off : off + grp, :])
                    for g in range(grp):
                        slab_map[off + g] = (a_t, g)

            # ---- matmul stream ----
            # OT = dS^T @ dA^T accumulated in four PSUM banks: bank (j, mc)
            # covers ot rows j*128..(j+1)*128, cols mc*512..(mc+1)*512.
            # Body pairs rotate j0mc0, j0mc1, j1mc0, j1mc1 (each lhsT serves
            # two consecutive matmuls); the last TAIL pairs run grouped per
            # bank so the banks stop staggered and evictions overlap the
            # remaining matmuls.
            ps = []
            for j in range(DOUT // P):
                ps_t = ps_pool.tile([P, ROWS], f32, name=f"ps{j}", tag=f"ps{j}")
                ps.append(ps_t)

            def mm(q, j, mc):
                t = 2 * q  # first slab of the pair
                a_t, g = slab_map[t]
                sc, sl = s_map[t]
                nc.tensor.matmul(
                    ps[j][:, mc * MM_N : (mc + 1) * MM_N],
                    sc[:, sl : sl + 2, j * P : (j + 1) * P],
                    a_t[:, g : g + 2, mc * MM_N : (mc + 1) * MM_N],
                    start=(q == 0),
                    stop=(q == KPAIRS - 1),
                    perf_mode=DR,
                )

            def evict(j, mc, n_out):
                # two half-width copy+DMA chains on both rings: the first
                # half's writeback starts while the second half copies, and
                # the two HBM write receipts at the very end run in parallel
                o_t = o_pool.tile(
                    [P, MM_N], f16, name=f"o_t{j}_{mc}", tag=f"o_t{j}_{mc}"
                )
                for half in range(2):
                    lo, hi = half * (MM_N // 2), (half + 1) * (MM_N // 2)
                    nc.vector.tensor_copy(
                        o_t[:, lo:hi], ps[j][:, mc * MM_N + lo : mc * MM_N + hi]
                    )
                    oeng = nc.sync if half == 0 else nc.scalar
                    oeng.dma_start(
                        out=ot[
                            j * P : (j + 1) * P, mc * MM_N + lo : mc * MM_N + hi
                        ],
                        in_=o_t[:, lo:hi],
                    )

            for q in range(KPAIRS - TAIL):
                for j in range(DOUT // P):
                    for mc in range(ROWS // MM_N):
                        mm(q, j, mc)
            n_out = 0
            for j in range(DOUT // P):
                for mc in range(ROWS // MM_N):
                    for q in range(KPAIRS - TAIL, KPAIRS):
                        mm(q, j, mc)
                    # bank (j, mc) is complete: evict to f16 and write back
                    # while the remaining banks' matmuls keep the PE busy
                    evict(j, mc, n_out)
                    n_out += 1

    nc.compile()
    _compiled = nc
    return _compiled


def kernel(input, weight):
    global last_results
    input = np.asarray(input, dtype=np.float32)
    weight = np.asarray(weight, dtype=np.float32)

    X = input[:, :NA]
    A = input[:, REALNA - 8 : REALNA - 8 + NSMP]  # [8192, 8192] view (no +I yet)

    # d = colsum(A + I); the identity adds exactly 1 to every column sum.
    d = A.sum(axis=0, dtype=np.float64) + 1.0
    dinv = (1.0 / np.sqrt(d)).astype(np.float32)  # [8192]
    # rowsum(dA) = rowsum(A + I) - a0*8192, needed for the mean correction
    rowsum_dA = (A.sum(axis=1, dtype=np.float64) + 1.0 - float(A0) * NSMP).astype(
        np.float32
    )

    support = X @ weight  # [8192, 256] f32
    S = support * dinv[:, None]
    mu = S.mean(axis=0, dtype=np.float64).astype(np.float32)  # [256]
    colsum_S = S.sum(axis=0, dtype=np.float64).astype(np.float32)  # [256]
    dS = (S - mu[None, :]).astype(ml_dtypes.float8_e4m3)
    # partition-major [128, 64, 256]
    s_dev = np.ascontiguousarray(dS.reshape(KSLABS, P, DOUT).swapaxes(0, 1))

    diag = np.arange(ROWS)
    in_maps = []
    for i in range(NCORES):
        blk = A[i * ROWS : (i + 1) * ROWS, :]  # [1024, 8192] view
        at_i = (blk.T - A0).astype(ml_dtypes.float8_e4m3)  # [8192, 1024]
        grows = i * ROWS + diag
        # fold the +I into this block's transposed, centered copy
        at_i[grows, diag] = (blk[diag, grows] + (1.0 - A0)).astype(
            ml_dtypes.float8_e4m3
        )
        # partition-major [128, 64, 1024]
        at_dev = np.ascontiguousarray(at_i.reshape(KSLABS, P, ROWS).swapaxes(0, 1))
        in_maps.append({"at": at_dev, "s": s_dev})

    # If BASS_TRACE is set but the axon NTFF hook module is absent, the
    # bass_utils trace path would die on import; provide a no-op hook so it
    # degrades to an untraced run instead.
    try:
        import antenv.axon_hooks  # noqa: F401
    except Exception:
        import sys
        import types

        _m = types.ModuleType("antenv.axon_hooks")
        _m.get_axon_ntff_profile_hook = lambda: None
        _m.set_axon_ntff_profile_hook = lambda h: None
        sys.modules["antenv.axon_hooks"] = _m

    from concourse.bass_utils import run_bass_kernel_spmd

    nc = _get_compiled()
    res = run_bass_kernel_spmd(nc, in_maps, list(range(NCORES)))
    last_results = res

    out = np.empty((NSMP, DOUT + NSMP), dtype=np.float32)
    out[:, DOUT:] = A
    gr = np.arange(NSMP)
    out[gr, DOUT + gr] += 1.0
    # exact mean terms: a0*colsum(S) + rowsum(dA)[:, None] * mu
    mean_terms = float(A0) * colsum_S[None, :] + rowsum_dA[:, None] * mu[None, :]
    for i in range(NCORES):
        ot_i = res.results[i]["ot"]  # [256, 1024] f16 = (dA_i @ dS)^T
        rows = slice(i * ROWS, (i + 1) * ROWS)
        out[rows, :DOUT] = (ot_i.T.astype(np.float32) + mean_terms[rows]) * dinv[
            rows, None
        ]
    return out
